# revision 20
# baseline (speedup 1.0000x reference)
"""Trainium2 Bass kernel for the decoder attention block (2x1024x1024, E=1024,
nhead=16, Tk=2048, F=4096, n_ctx mask over first keys).

Sharding: 8 NeuronCores = 2 batches x 4 query-token ranges (256 rows each);
weights replicated and streamed from HBM; per-core self/cross K+V.

v2: fp8(e4m3) DoubleRow matmuls (0.5 cycles/row) for all QKV projections and
the attention AV contraction; bf16 for scores, out_proj and FFN; fp32 PSUM
throughout and fp32 residual stream. QKV weights are pre-scaled x32 on the
host before the fp8 cast (folded back via the exp scale on the scores path
and via the 1/32 ones-row on the AV denominator path). Out-proj runs with
head-pairs packed into the full K=128 contraction; softmax normalization is
batched per head-pair; cross-attention K/V projection is interleaved with the
per-chunk softmax so the PE stays busy while the scalar engine runs exp; FFN
weights are prefetched during cross attention.

Self-contained: builds the Bass/Tile program, shards the full inputs on the
host, runs SPMD on cores 0-7 via run_bass_kernel_spmd, reassembles the output.
"""
import sys
if "/opt/trn_rl_repo" not in sys.path:
    sys.path.insert(0, "/opt/trn_rl_repo")


from contextlib import ExitStack

import numpy as np

import concourse.bass as bass
import concourse.mybir as mybir
import concourse.tile as tile
from concourse import bacc
from concourse.masks import make_identity

f32 = mybir.dt.float32
f32r = mybir.dt.float32r
bf16 = mybir.dt.bfloat16
f8 = mybir.dt.float8e4
AF = mybir.ActivationFunctionType
DR = mybir.MatmulPerfMode.DoubleRow

P = 128
E = 1024
EC = E // P            # 8 feature chunks
KP = EC // 2           # 4 DoubleRow feature-pair chunks
TQ = 256               # query tokens per core
TQT = TQ // P          # 2
TM = 2048              # cross-attention memory tokens
F = 4096
FC = F // P            # 32
H = 16
HP = H // 2            # 8 head pairs
Dh = 64
EPS = 1e-5
CHUNK = 512            # kv processing chunk (tokens)
CT = CHUNK // P        # 4 tiles per chunk
WS = 32.0              # host-side fp8 weight prescale
EXP_SCALE = 0.125 / (WS * WS)


def _r(ap):
    return ap.bitcast(f32r) if ap.dtype == f32 else ap


def build_nc(n_ctx: int, loop_n: int = 0):
    """Build the single-core SPMD program. n_ctx: self-attn context length."""
    uniform_self = n_ctx == 0
    n_ctx_eff = 1024 if uniform_self else int(n_ctx)
    TC = (n_ctx_eff + P - 1) // P     # context tiles
    TCTX = TC * P
    rem = n_ctx_eff - (TC - 1) * P    # valid rows in last tile (1..128)
    need_mask = (rem != P) and not uniform_self

    nc = bacc.Bacc("TRN2", target_bir_lowering=False, debug=False)

    # ---------------- DRAM parameters ----------------
    xq_d = nc.declare_dram_parameter("xq", [TQ, E], f32, isOutput=False)
    xc_d = nc.declare_dram_parameter("xc", [TCTX, E], f32, isOutput=False)
    memT_d = nc.declare_dram_parameter("memT", [P, KP, 2, TM], f8, isOutput=False)
    w8_names = ["s_wq", "s_wk", "s_wv", "c_wq", "c_wk", "c_wv"]
    w8d = {n: nc.declare_dram_parameter(n, [P, KP, 2, E], f8, isOutput=False)
           for n in w8_names}
    sowP_d = nc.declare_dram_parameter("s_owP", [P, HP, E], bf16, isOutput=False)
    cowP_d = nc.declare_dram_parameter("c_owP", [P, HP, E], bf16, isOutput=False)
    w1T_d = nc.declare_dram_parameter("w1T", [E, F], bf16, isOutput=False)
    w2T_d = nc.declare_dram_parameter("w2T", [F, E], bf16, isOutput=False)
    cmask_d = nc.declare_dram_parameter("cmask", [P, TC], f32, isOutput=False)
    out_d = nc.declare_dram_parameter("out", [TQ, E], f32, isOutput=True)

    xq_r = xq_d.rearrange("(c p) e -> p c e", p=P)        # [128, TQT, E]
    xc_r = xc_d.rearrange("(c p) e -> p c e", p=P)        # [128, TC, E]
    w1T_r = w1T_d.rearrange("(c p) m -> p c m", p=P)      # [128, EC, F]
    w2T_r = w2T_d.rearrange("(c p) m -> p c m", p=P)      # [128, FC, E]
    out_r = out_d.rearrange("(c p) e -> p c e", p=P)

    ctx = ExitStack()
    with ctx:
        ctx.enter_context(nc.allow_low_precision(reason="fp8/bf16 matmul intended"))
        tc = ctx.enter_context(tile.TileContext(nc))

        # ---- kernel-lifetime pools ----
        const = ctx.enter_context(tc.tile_pool(name="const", bufs=1))
        xpool = ctx.enter_context(tc.tile_pool(name="x", bufs=1))
        wq8pool = ctx.enter_context(tc.tile_pool(name="w8", bufs=6))
        cwkvpool = ctx.enter_context(tc.tile_pool(name="cwkv", bufs=4))
        owpool = ctx.enter_context(tc.tile_pool(name="ow", bufs=2))
        ffn1pool = ctx.enter_context(tc.tile_pool(name="ffn1", bufs=3))
        spool = ctx.enter_context(tc.tile_pool(name="stats", bufs=6))
        rdtpool = ctx.enter_context(tc.tile_pool(name="rdt", bufs=3))
        rdpool = ctx.enter_context(tc.tile_pool(name="rd", bufs=1))
        # psum: 5 + 3 = 8 banks
        ps5 = ctx.enter_context(tc.tile_pool(name="ps5", bufs=4, space="PSUM"))
        psS = ctx.enter_context(tc.tile_pool(name="psS", bufs=2, space="PSUM"))

        # ---- constants ----
        identb = const.tile([P, P], bf16)
        make_identity(nc, identb)
        ones_r = const.tile([1, Dh], bf16)
        nc.vector.memset(ones_r[:], 1.0 / WS)
        onesv = const.tile([P, CT, H], f8)
        nc.vector.memset(onesv[:], 1.0)
        onesq = const.tile([P, TQ], f8)
        if uniform_self:
            nc.vector.memset(onesq[:], 1.0)
        eps_t = const.tile([P, 1], f32)
        nc.vector.memset(eps_t[:], EPS)
        cmask_t = const.tile([P, TC], f32)
        if need_mask:
            nc.sync.dma_start(out=cmask_t[:], in_=cmask_d[:])

        if loop_n:
            ctx.enter_context(tc.For_i(0, loop_n, 1))

        # ---- persistent residual state ----
        x_q = xpool.tile([P, TQT, E], f32, tag="xq")
        nc.sync.dma_start(out=x_q[:], in_=xq_r[:])

        # ---------------- helpers ----------------
        def load_w8h(name, half, nm, pool):
            """fp8 DR weight half: [128, KP, 2, 512] (cols half*512...)."""
            t = pool.tile([P, KP, 2, 512], f8, tag="w8", name=nm)
            nc.sync.dma_start(
                out=t[:], in_=w8d[name][:, :, :, half * 512:(half + 1) * 512])
            return t

        def load_w8(name, nm, pool=None):
            pool = pool or wq8pool
            return [load_w8h(name, 0, nm + "a", pool),
                    load_w8h(name, 1, nm + "b", pool)]

        def w8col(wpair, c0):
            """[128, 2, 128] DR stationary slice at out-col c0 (needs kp idx)."""
            h, off = divmod(c0, 512)
            return wpair[h], off

        def layernorm(x_tiles, n_tiles, out_tile):
            """token-major LN: out = (x - mean) * rsqrt(var + eps)."""
            for t in range(n_tiles):
                st = spool.tile([P, 2, 6], f32, tag="bnst")
                xin = x_tiles[:, t, :].rearrange("p (s e) -> p s e", s=2)
                for s in range(2):
                    nc.vector.bn_stats(out=st[:, s, :], in_=xin[:, s, :])
                mv = spool.tile([P, 2], f32, tag="bnmv")
                nc.vector.bn_aggr(out=mv[:], in_=st[:])
                sd = spool.tile([P, 1], f32, tag="bnsd")
                nc.scalar.activation(sd[:], mv[:, 1:2], AF.Sqrt, bias=eps_t[:])
                rstd = spool.tile([P, 1], f32, tag="bnrs")
                nc.vector.reciprocal(rstd[:], sd[:])
                nm = spool.tile([P, 1], f32, tag="bnnm")
                nc.vector.tensor_mul(nm[:], mv[:, 0:1], rstd[:])
                nc.scalar.mul(nm[:], nm[:], -1.0)
                nc.scalar.activation(out_tile[:, t, :], x_tiles[:, t, :], AF.Identity,
                                     bias=nm[:], scale=rstd[:])

        def transpose_in(nx_tiles, n_tiles, outT, ident):
            """(128, n_tiles, E) token-major -> (128, EC, n_tiles*128) feat-major.
            dtype of nx/outT/ident must match (fp8 or bf16)."""
            dt_ = outT.dtype if hasattr(outT, "dtype") else f8
            for e in range(EC):
                for t0 in range(0, n_tiles, 4):
                    tn = min(4, n_tiles - t0)
                    pst = ps5.tile([P, 512], ident.dtype, tag="ps5")
                    for i in range(tn):
                        nc.tensor.transpose(
                            pst[:, i * P:(i + 1) * P],
                            nx_tiles[:, t0 + i, e * P:(e + 1) * P], ident[:])
                    nc.scalar.activation(outT[:, e, t0 * P:(t0 + tn) * P],
                                         pst[:, 0:tn * P], AF.Copy)

        def proj_dr(src_pair, src_cols, wpair, outT, eng="scalar"):
            """Feature-major DR projection.
            src_pair(kp, cols) -> [128, 2, cols] moving slice for feat-pair kp.
            outT[:, m, :cols] = (w.T x)."""
            for m in range(EC):
                ps = ps5.tile([P, 512], f32, tag="ps5")
                wh, off = w8col(wpair, m * P)
                for kp in range(KP):
                    nc.tensor.matmul(ps[:, 0:src_cols],
                                     wh[:, kp, :, off:off + P],
                                     src_pair(kp, src_cols),
                                     start=(kp == 0), stop=(kp == KP - 1),
                                     perf_mode=DR)
                copy_out(eng, outT[:, m, 0:src_cols], ps[:, 0:src_cols])

        def copy_out(eng, dst, src):
            if eng == "scalar":
                nc.scalar.activation(dst, src, AF.Copy)
            else:
                nc.vector.tensor_copy(dst, src)

        def kv_project_gen(srcT_pair, src_tok_pair, c0, ct, wk, wv, kTc, vpk,
                           dst0=0, eng="vector"):
            """Generator: one PSUM group per next(). K into kTc (bf16
            feat-major) at col dst0, V into vpk (fp8 token-major + ones col)
            at tile dst0//P.
            srcT_pair(kp, c0, n) -> [128, 2, n] moving (features x tokens);
            src_tok_pair(kp, t0) -> [128, 2, 128] stationary (feature x token)."""
            dt0 = dst0 // P
            nc.vector.tensor_copy(vpk[:, dt0:dt0 + ct, :, 64], onesv[:, 0:ct, :])
            for m in range(EC):
                ps = ps5.tile([P, 512], f32, tag="ps5")
                wh, off = w8col(wk, m * P)
                for kp in range(KP):
                    nc.tensor.matmul(ps[:, 0:ct * P],
                                     wh[:, kp, :, off:off + P],
                                     srcT_pair(kp, c0, ct * P),
                                     start=(kp == 0), stop=(kp == KP - 1),
                                     perf_mode=DR)
                copy_out(eng, kTc[:, m, dst0:dst0 + ct * P], ps[:, 0:ct * P])
                yield
            for mt in range(ct):
                for half in range(2):
                    ps = ps5.tile([P, 512], f32, tag="ps5")
                    for kp in range(KP):
                        nc.tensor.matmul(
                            ps[:],
                            src_tok_pair(kp, c0 + mt * P),
                            wv[half][:, kp, :, :],
                            start=(kp == 0), stop=(kp == KP - 1),
                            perf_mode=DR)
                    copy_out(eng,
                             vpk[:, dt0 + mt, half * 8:(half + 1) * 8, 0:64],
                             ps.rearrange("p (h d) -> p h d", d=64))
                    yield

        def kv_project(*args, **kwargs):
            for _ in kv_project_gen(*args, **kwargs):
                pass

        def attn_chunk(qT, kTc, vpk, ct, acc65, ppool, first, last, masked,
                       uniform, kc0=0, filler=None):
            """One kv chunk of attention for all heads; accumulate into acc65.
            kc0: column offset into kTc (and tile offset kc0//P into vpk).
            filler(h): emits extra PE work between scores and AV."""
            vt0 = kc0 // P
            for h in range(H):
                ht, hr = h // 2, (h % 2) * Dh
                probsT = ppool.tile([P, CT, TQ], f8, tag="probs")
                if not uniform:
                    ps_s = psS.tile([P, CT, TQ], f32, tag="psS")
                    for kt in range(ct):
                        nc.tensor.matmul(
                            ps_s[:, kt, :],
                            kTc[hr:hr + Dh, ht, kc0 + kt * P:kc0 + (kt + 1) * P],
                            qT[hr:hr + Dh, ht, :],
                            start=True, stop=True)
                    nc.scalar.activation(probsT[:, 0:ct, :],
                                         ps_s[:, 0:ct, :], AF.Exp,
                                         scale=EXP_SCALE)
                    if masked and last:
                        nc.vector.tensor_scalar_mul(
                            probsT[:, ct - 1, :], probsT[:, ct - 1, :],
                            cmask_t[:, TC - 1:TC])
                else:
                    for kt in range(ct):
                        nc.vector.tensor_copy(probsT[:, kt, :], onesq[:])
                if filler is not None:
                    filler(h)
                ps_av = ps5.tile([65, TQ], f32, tag="ps5")
                nkt = ct // 2
                for ktp in range(nkt):
                    nc.tensor.matmul(ps_av[:],
                                     vpk[:, vt0 + 2 * ktp:vt0 + 2 * ktp + 2, h, :],
                                     probsT[:, 2 * ktp:2 * ktp + 2, :],
                                     start=(ktp == 0),
                                     stop=(ktp == nkt - 1 and ct % 2 == 0),
                                     perf_mode=DR)
                if ct % 2:
                    nc.tensor.matmul(ps_av[:],
                                     vpk[:, vt0 + ct - 1, h, :],
                                     probsT[:, ct - 1, :],
                                     start=(ct == 1), stop=True)
                if first:
                    nc.vector.tensor_copy(acc65[:, h, :], ps_av[:])
                else:
                    nc.vector.tensor_add(acc65[:, h, :], acc65[:, h, :], ps_av[:])

        def attn_super(qT, kv0, kv1, acc65, ppool, first, filler=None):
            """Two cross kv chunks per acc65 update (CT tiles each, no mask)."""
            kTc0, vpk0 = kv0
            kTc1, vpk1 = kv1
            for h in range(H):
                ht, hr = h // 2, (h % 2) * Dh
                probs = []
                for kTc in (kTc0, kTc1):
                    probsT = ppool.tile([P, CT, TQ], f8, tag="probs")
                    ps_s = psS.tile([P, CT, TQ], f32, tag="psS")
                    for kt in range(CT):
                        nc.tensor.matmul(
                            ps_s[:, kt, :],
                            kTc[hr:hr + Dh, ht, kt * P:(kt + 1) * P],
                            qT[hr:hr + Dh, ht, :],
                            start=True, stop=True)
                    nc.scalar.activation(probsT[:, 0:CT, :],
                                         ps_s[:, 0:CT, :], AF.Exp,
                                         scale=EXP_SCALE)
                    if filler is not None:
                        filler(h)
                    probs.append(probsT)
                ps_av = ps5.tile([65, TQ], f32, tag="ps5")
                nsub = CT // 2
                for sub in range(2):
                    vpk, probsT = (vpk0, probs[0]) if sub == 0 else (vpk1, probs[1])
                    for ktp in range(nsub):
                        nc.tensor.matmul(ps_av[:],
                                         vpk[:, 2 * ktp:2 * ktp + 2, h, :],
                                         probsT[:, 2 * ktp:2 * ktp + 2, :],
                                         start=(sub == 0 and ktp == 0),
                                         stop=(sub == 1 and ktp == nsub - 1),
                                         perf_mode=DR)
                if first:
                    nc.vector.tensor_copy(acc65[:, h, :], ps_av[:])
                else:
                    nc.vector.tensor_add(acc65[:, h, :], acc65[:, h, :], ps_av[:])

        def normalize(acc65, nm):
            """acc65 [65, H, TQ] -> accP [128, HP, TQ] bf16 pair-packed
            (odd heads land in partitions 64-127 via SBUF->SBUF DMA).
            Folds the 1/WS V-scale via ones_r."""
            accP = rdpool.tile([P, HP, TQ], bf16, tag="accb", name=f"{nm}ab")
            rds = []
            for t in range(HP):
                rd = rdtpool.tile([1, 2, TQ], bf16, tag="rdt", name=f"{nm}rd{t}")
                nc.vector.reciprocal(rd[:], acc65[64:65, 2 * t:2 * t + 2, :])
                rds.append(rd)
            for t in range(HP):
                psb = psS.tile([Dh, 2, TQ], f32, tag="psS", name=f"{nm}psb{t}")
                nc.tensor.matmul(psb[:, 0, :].rearrange("p a b -> p (a b)")
                                 if False else psb[:],
                                 ones_r[:], rds[t][:],
                                 start=True, stop=True)
                nc.vector.tensor_mul(accP[0:64, t, :],
                                     acc65[0:64, 2 * t, :], psb[:, 0, :])
                odd = rdtpool.tile([Dh, TQ], bf16, tag="odd", name=f"{nm}od{t}")
                nc.vector.tensor_mul(odd[:],
                                     acc65[0:64, 2 * t + 1, :], psb[:, 1, :])
                nc.sync.dma_start(out=accP[64:128, t, :], in_=odd[:])
            return accP

        def out_proj(accP, owP_dram, dest, nm):
            """dest[:, tq, :] += attn @ ow; head-pairs packed, K=128."""
            for nq in range(4):
                owq = owpool.tile([P, HP, 256], bf16, tag="ow", name=f"{nm}{nq}")
                nc.sync.dma_start(out=owq[:],
                                  in_=owP_dram[:, :, nq * 256:(nq + 1) * 256])
                for tq in range(TQT):
                    ps = psS.tile([P, TQ], f32, tag="psS")
                    for hp in range(HP):
                        nc.tensor.matmul(
                            ps[:],
                            accP[:, hp, tq * P:(tq + 1) * P],
                            owq[:, hp, :],
                            start=(hp == 0), stop=(hp == HP - 1))
                    nc.vector.tensor_add(dest[:, tq, nq * 256:(nq + 1) * 256],
                                         dest[:, tq, nq * 256:(nq + 1) * 256],
                                         ps[:])

        # =========================================================
        # Phase 1: LN1 + transposes (fp8), self QKV, cross KV c0/c1
        mpool = ctx.enter_context(tc.tile_pool(name="mem", bufs=3))
        kTpool = ctx.enter_context(tc.tile_pool(name="kT", bufs=4))
        vpool = ctx.enter_context(tc.tile_pool(name="v", bufs=4))

        cross_kv = {}

        def emit_cross_kv(c, eng="vector"):
            mt_ = mpool.tile([P, KP, 2, CHUNK], f8, tag="mem", name=f"mem{c}")
            nc.sync.dma_start(
                out=mt_[:],
                in_=memT_d[:, :, :, c * CHUNK:(c + 1) * CHUNK])
            kTc = kTpool.tile([P, EC, CHUNK], bf16, tag="kT", name=f"ckT{c}")
            vpk = vpool.tile([P, CT, H, 65], f8, tag="v", name=f"cv{c}")
            cross_kv[c] = (kTc, vpk)
            return kv_project_gen(
                lambda kp, c0, n: mt_[:, kp, :, c0:c0 + n],
                lambda kp, t0: mt_[:, kp, :, t0:t0 + P],
                0, CT, cwk, cwv, kTc, vpk, eng=eng)

        swq = load_w8("s_wq", "swq")
        with tc.tile_pool(name="nxT1", bufs=1) as nxT1:
            nxqT = nxT1.tile([P, EC, TQ], f8, tag="nxqT")
            nxcT = nxT1.tile([P, EC, TCTX], f8, tag="nxcT")
            with tc.tile_pool(name="p1", bufs=1) as p1:
                xc_t = p1.tile([P, TC, E], f32, tag="xc")
                nc.sync.dma_start(out=xc_t[:], in_=xc_r[:])
                nx_q = p1.tile([P, TQT, E], bf16, tag="nxq")
                layernorm(x_q, TQT, nx_q)
                transpose_in(nx_q, TQT, nxqT, identb)
                nx_c = p1.tile([P, TC, E], bf16, tag="nxc")
                layernorm(xc_t, TC, nx_c)
                transpose_in(nx_c, TC, nxcT, identb)

            # Phase 2: self QKV + attention + out_proj
            cwk = load_w8("c_wk", "cwk", cwkvpool)
            cwv = load_w8("c_wv", "cwv", cwkvpool)
            with tc.tile_pool(name="qT2", bufs=1) as qTp, \
                 tc.tile_pool(name="skT", bufs=1) as skTp, \
                 tc.tile_pool(name="sv", bufs=1) as svp, \
                 tc.tile_pool(name="pr2", bufs=3) as pp, \
                 tc.tile_pool(name="at2", bufs=1) as ap_:
                qT = qTp.tile([P, EC, TQ], bf16, tag="qT")
                proj_dr(lambda kp, n: nxqT[:, 2 * kp:2 * kp + 2, 0:n], TQ,
                        swq, qT)
                swk = load_w8("s_wk", "swk")
                swv = load_w8("s_wv", "swv")
                kTc_s = skTp.tile([P, EC, TCTX], bf16, tag="skT")
                vpk_s = svp.tile([P, TC, H, 65], f8, tag="sv")
                # self K/V over the context tiles (chunks of <=CT tiles)
                c0 = 0
                while c0 < TCTX:
                    ct = min(CT, (TCTX - c0) // P)
                    kv_project(
                        lambda kp, cc, n: nxcT[:, 2 * kp:2 * kp + 2, cc:cc + n],
                        lambda kp, t0: nxcT[:, 2 * kp:2 * kp + 2, t0:t0 + P],
                        c0, ct, swk, swv, kTc_s, vpk_s, dst0=c0)
                    c0 += ct * P
                # cross kv chunk 0 interleaved into self attention
                kv0 = emit_cross_kv(0)
                acc65 = ap_.tile([65, H, TQ], f32r, tag="acc65")
                n_sc = (TCTX + CHUNK - 1) // CHUNK
                for ci in range(n_sc):
                    cc0 = ci * CHUNK
                    ct = min(CT, (TCTX - cc0) // P)
                    attn_chunk(qT, kTc_s, vpk_s, ct, acc65, pp,
                               ci == 0, ci == n_sc - 1, need_mask, uniform_self,
                               kc0=cc0,
                               filler=(lambda h: next(kv0, None)) if ci == 0
                               else None)
                for _ in kv0:
                    pass
                for _ in emit_cross_kv(1, eng="scalar"):
                    pass
                accb_s = normalize(acc65, "s")
                out_proj(accb_s, sowP_d, x_q, "sow")

        # Phase 3/4: LN2 + transpose, cross q + attention + out_proj
        with tc.tile_pool(name="qT4", bufs=1) as qTp, \
             tc.tile_pool(name="pr4", bufs=3) as pp, \
             tc.tile_pool(name="at4", bufs=1) as ap_:
            with tc.tile_pool(name="nxT3", bufs=1) as nxT3:
                nx2T = nxT3.tile([P, EC, TQ], f8, tag="nx2T")
                with tc.tile_pool(name="p3", bufs=1) as p3:
                    nx2 = p3.tile([P, TQT, E], bf16, tag="nx2")
                    layernorm(x_q, TQT, nx2)
                    transpose_in(nx2, TQT, nx2T, identb)
                cwq = load_w8("c_wq", "cwq")
                qT = qTp.tile([P, EC, TQ], bf16, tag="qT")
                proj_dr(lambda kp, n: nx2T[:, 2 * kp:2 * kp + 2, 0:n], TQ,
                        cwq, qT, eng="vector")
            acc65 = ap_.tile([65, H, TQ], f32r, tag="acc65")
            n_cc = TM // CHUNK
            assert n_cc % 2 == 0
            for sc in range(n_cc // 2):
                gens = [emit_cross_kv(cc) for cc in (2 * sc + 2, 2 * sc + 3)
                        if cc < n_cc]
                if sc == 0:
                    for g in range(2):
                        w1g = ffn1pool.tile([P, EC, 512], bf16, tag="ffn1",
                                            name=f"w1g{g}")
                        nc.sync.dma_start(
                            out=w1g[:], in_=w1T_r[:, :, g * 512:(g + 1) * 512])
                        cross_kv[f"w1_{g}"] = w1g

                def filler(h, gens=gens):
                    for g in gens:
                        if next(g, None) is not None:
                            return
                kv0 = cross_kv.pop(2 * sc)
                kv1 = cross_kv.pop(2 * sc + 1)
                attn_super(qT, kv0, kv1, acc65, pp, sc == 0,
                           filler=filler if gens else None)
                for g in gens:
                    for _ in g:
                        pass
            accb_c = normalize(acc65, "c")
            out_proj(accb_c, cowP_d, x_q, "cow")

        # Phase 5: LN3 + transpose (bf16); Phase 6: FFN
        with tc.tile_pool(name="nxT5", bufs=1) as nxT5:
            nx3T = nxT5.tile([P, EC, TQ], bf16, tag="nx3T")
            with tc.tile_pool(name="p5", bufs=1) as p5:
                nx3 = p5.tile([P, TQT, E], bf16, tag="nx3")
                layernorm(x_q, TQT, nx3)
                transpose_in(nx3, TQT, nx3T, identb)

            with tc.tile_pool(name="hT", bufs=1) as hp_, \
                 tc.tile_pool(name="ffn2", bufs=3) as ffn2pool:
                hT = hp_.tile([P, FC, TQ], bf16, tag="hT")
                w2pre = {}
                for g in range(2):
                    w2g = ffn2pool.tile([P, 4, E], bf16, tag="ffn2",
                                        name=f"w2g{g}")
                    nc.sync.dma_start(out=w2g[:],
                                      in_=w2T_r[:, g * 4:(g + 1) * 4, :])
                    w2pre[g] = w2g
                for g in range(8):
                    if f"w1_{g}" in cross_kv:
                        w1g = cross_kv.pop(f"w1_{g}")
                    else:
                        w1g = ffn1pool.tile([P, EC, 512], bf16, tag="ffn1",
                                            name=f"w1g{g}")
                        nc.sync.dma_start(
                            out=w1g[:], in_=w1T_r[:, :, g * 512:(g + 1) * 512])
                    for m in range(4):
                        ps = ps5.tile([P, 512], f32, tag="ps5")
                        for k in range(EC):
                            nc.tensor.matmul(ps[:, 0:TQ],
                                             w1g[:, k, m * P:(m + 1) * P],
                                             nx3T[:, k, :],
                                             start=(k == 0), stop=(k == EC - 1))
                        nc.scalar.activation(hT[:, g * 4 + m, :], ps[:, 0:TQ],
                                             AF.Gelu)
                # y = hT.T @ w2T, accumulated over all 32 k-tiles
                ps_y = [[ps5.tile([P, 512], f32, tag="ps5", name=f"psy_{tq}_{nh}")
                         for nh in range(2)] for tq in range(TQT)]
                for g in range(8):
                    if g in w2pre:
                        w2g = w2pre.pop(g)
                    else:
                        w2g = ffn2pool.tile([P, 4, E], bf16, tag="ffn2",
                                            name=f"w2g{g}")
                        nc.sync.dma_start(out=w2g[:],
                                          in_=w2T_r[:, g * 4:(g + 1) * 4, :])
                    for tq in range(TQT):
                        for nh in range(2):
                            for k in range(4):
                                kk = g * 4 + k
                                nc.tensor.matmul(
                                    ps_y[tq][nh][:],
                                    hT[:, kk, tq * P:(tq + 1) * P],
                                    w2g[:, k, nh * 512:(nh + 1) * 512],
                                    start=(kk == 0), stop=(kk == FC - 1))
                for tq in range(TQT):
                    for nh in range(2):
                        nc.vector.tensor_add(x_q[:, tq, nh * 512:(nh + 1) * 512],
                                             x_q[:, tq, nh * 512:(nh + 1) * 512],
                                             ps_y[tq][nh][:])

        nc.sync.dma_start(out=out_r[:], in_=x_q[:])

    nc.finalize()
    return nc


# ======================= host side =======================

def host_prep(inputs):
    """Fold LN affine into weights; fp8/bf16 layouts; slice per core.
    Returns (n_ctx, in_maps list of 8 dicts)."""
    import ml_dtypes
    F8 = ml_dtypes.float8_e4m3
    BF = ml_dtypes.bfloat16

    tgt = np.asarray(inputs["tgt"], np.float32)
    memory = np.asarray(inputs["memory"], np.float32)
    n_ctx = int(np.asarray(inputs["n_ctx"]))
    ln1_g = np.asarray(inputs["ln1_g"], np.float32)
    ln1_b = np.asarray(inputs["ln1_b"], np.float32)
    ln2_g = np.asarray(inputs["ln2_g"], np.float32)
    ln2_b = np.asarray(inputs["ln2_b"], np.float32)
    ln3_g = np.asarray(inputs["ln3_g"], np.float32)
    ln3_b = np.asarray(inputs["ln3_b"], np.float32)

    def fold(w, b, g, lb):
        w_eff = w * g[None, :]
        b_eff = w @ lb + b
        return w_eff, b_eff

    s_w = np.asarray(inputs["self_w"], np.float32)
    s_b = np.asarray(inputs["self_b"], np.float32)
    c_w = np.asarray(inputs["cross_w"], np.float32)
    c_b = np.asarray(inputs["cross_b"], np.float32)
    s_ow = np.asarray(inputs["self_ow"], np.float32)
    s_ob = np.asarray(inputs["self_ob"], np.float32)
    c_ow = np.asarray(inputs["cross_ow"], np.float32)
    c_ob = np.asarray(inputs["cross_ob"], np.float32)
    w1 = np.asarray(inputs["w1"], np.float32)
    b1 = np.asarray(inputs["b1"], np.float32)
    w2 = np.asarray(inputs["w2"], np.float32)
    b2 = np.asarray(inputs["b2"], np.float32)

    s_wq, s_bq = fold(s_w[:E], s_b[:E], ln1_g, ln1_b)
    s_wk, s_bk = fold(s_w[E:2 * E], s_b[E:2 * E], ln1_g, ln1_b)
    s_wv, s_bv = fold(s_w[2 * E:], s_b[2 * E:], ln1_g, ln1_b)
    c_wq, c_bq = fold(c_w[:E], c_b[:E], ln2_g, ln2_b)
    c_wk, c_bk = c_w[E:2 * E], c_b[E:2 * E]
    c_wv, c_bv = c_w[2 * E:], c_b[2 * E:]
    w1_eff, b1_eff = fold(w1, b1, ln3_g, ln3_b)

    zero_bias = not (np.any(s_bq) or np.any(s_bk) or np.any(s_bv) or np.any(s_ob)
                     or np.any(c_bq) or np.any(c_bk) or np.any(c_bv) or np.any(c_ob)
                     or np.any(b1_eff) or np.any(b2))
    if not zero_bias:
        raise NotImplementedError("nonzero biases not supported by this kernel")

    uniform = n_ctx == 0
    n_ctx_eff = 1024 if uniform else n_ctx
    TC = (n_ctx_eff + P - 1) // P
    TCTX = TC * P

    def w8dr(w):
        """[E_out, E_in] weight -> fp8 DR layout [128, KP, 2, E_out], x WS."""
        wT = np.ascontiguousarray(w.T) * WS                  # [E_in, E_out]
        return np.ascontiguousarray(
            wT.reshape(KP, 2, P, w.shape[0]).transpose(2, 0, 1, 3)).astype(F8)

    def owpair(ow):
        """[E, E] out-proj -> bf16 pair layout [128, HP, E]."""
        owT = np.ascontiguousarray(ow.T)                     # [E_in(hd), E_out]
        return np.ascontiguousarray(
            owT.reshape(HP, 2, Dh, E).transpose(1, 2, 0, 3).reshape(P, HP, E)
        ).astype(BF)

    def b16(a):
        return np.ascontiguousarray(a.T).astype(BF)

    def memdr(m):
        """[Tk, E] memory -> fp8 DR layout [128, KP, 2, Tk]."""
        mT = np.ascontiguousarray(m.T)                       # [E, Tk]
        return np.ascontiguousarray(
            mT.reshape(KP, 2, P, m.shape[0]).transpose(2, 0, 1, 3)).astype(F8)

    shared = {
        "s_wq": w8dr(s_wq),
        "s_wk": w8dr(s_wk),
        "s_wv": w8dr(s_wv),
        "c_wq": w8dr(c_wq),
        "c_wk": w8dr(c_wk),
        "c_wv": w8dr(c_wv),
        "s_owP": owpair(s_ow),
        "c_owP": owpair(c_ow),
        "w1T": b16(w1_eff),
        "w2T": b16(w2),
    }
    cmask = np.ones((P, TC), np.float32)
    nvalid = n_ctx_eff - (TC - 1) * P
    cmask[nvalid:, TC - 1] = 0.0

    memT = [memdr(memory[g]) for g in range(2)]
    xcs = []
    for g in range(2):
        xc = tgt[g, :min(n_ctx_eff, 1024)]
        if xc.shape[0] < TCTX:
            xc = np.concatenate([xc, np.zeros((TCTX - xc.shape[0], E), np.float32)], 0)
        xcs.append(np.ascontiguousarray(xc))

    in_maps = []
    for c in range(8):
        g, r0 = c // 4, (c % 4) * TQ
        m = dict(shared)
        m["memT"] = memT[g]
        m["xq"] = np.ascontiguousarray(tgt[g, r0:r0 + TQ])
        m["xc"] = xcs[g]
        m["cmask"] = cmask
        in_maps.append(m)
    return n_ctx, in_maps


def assemble(results):
    out = np.empty((2, 1024, E), np.float32)
    for c in range(8):
        g, r0 = c // 4, (c % 4) * TQ
        out[g, r0:r0 + TQ] = results[c]["out"]
    return out


_NC_CACHE = {}


def kernel(**inputs):
    """Full (unsharded) inputs -> full (2, 1024, 1024) float32 output."""
    from concourse.bass_utils import run_bass_kernel_spmd
    n_ctx, in_maps = host_prep(inputs)
    nc = _NC_CACHE.get(n_ctx)
    if nc is None:
        nc = build_nc(n_ctx)
        _NC_CACHE[n_ctx] = nc
    res = run_bass_kernel_spmd(nc, in_maps, list(range(8)))
    return assemble(res.results)


# revision 21
# speedup vs baseline: 1.0942x; 1.0942x over previous
"""Trainium2 Bass kernel for the decoder attention block (2x1024x1024, E=1024,
nhead=16, Tk=2048, F=4096, n_ctx mask over first keys).

Sharding: 8 NeuronCores = 2 batches x 4 query-token ranges (256 rows each);
weights replicated and streamed from HBM; per-core self/cross K+V.

v2: fp8(e4m3) DoubleRow matmuls (0.5 cycles/row) for all QKV projections and
the attention AV contraction; bf16 for scores, out_proj and FFN; fp32 PSUM
throughout and fp32 residual stream. QKV weights are pre-scaled x32 on the
host before the fp8 cast (folded back via the exp scale on the scores path
and via the 1/32 ones-row on the AV denominator path). Out-proj runs with
head-pairs packed into the full K=128 contraction; softmax normalization is
batched per head-pair; cross-attention K/V projection is interleaved with the
per-chunk softmax so the PE stays busy while the scalar engine runs exp; FFN
weights are prefetched during cross attention.

Self-contained: builds the Bass/Tile program, shards the full inputs on the
host, runs SPMD on cores 0-7 via run_bass_kernel_spmd, reassembles the output.
"""
import sys
if "/opt/trn_rl_repo" not in sys.path:
    sys.path.insert(0, "/opt/trn_rl_repo")


from contextlib import ExitStack

import numpy as np

import concourse.bass as bass
import concourse.mybir as mybir
import concourse.tile as tile
from concourse import bacc
from concourse.masks import make_identity

f32 = mybir.dt.float32
f32r = mybir.dt.float32r
bf16 = mybir.dt.bfloat16
f8 = mybir.dt.float8e4
AF = mybir.ActivationFunctionType
DR = mybir.MatmulPerfMode.DoubleRow

P = 128
E = 1024
EC = E // P            # 8 feature chunks
KP = EC // 2           # 4 DoubleRow feature-pair chunks
TQ = 256               # query tokens per core
TQT = TQ // P          # 2
TM = 2048              # cross-attention memory tokens
F = 4096
FC = F // P            # 32
H = 16
HP = H // 2            # 8 head pairs
Dh = 64
EPS = 1e-5
CHUNK = 512            # kv processing chunk (tokens)
CT = CHUNK // P        # 4 tiles per chunk
WS = 32.0              # host-side fp8 weight prescale
EXP_SCALE = 0.125 / (WS * WS)


def _r(ap):
    return ap.bitcast(f32r) if ap.dtype == f32 else ap


def build_nc(n_ctx: int, loop_n: int = 0):
    """Build the single-core SPMD program. n_ctx: self-attn context length."""
    uniform_self = n_ctx == 0
    n_ctx_eff = 1024 if uniform_self else int(n_ctx)
    TC = (n_ctx_eff + P - 1) // P     # context tiles
    TCTX = TC * P
    rem = n_ctx_eff - (TC - 1) * P    # valid rows in last tile (1..128)
    need_mask = (rem != P) and not uniform_self

    nc = bacc.Bacc("TRN2", target_bir_lowering=False, debug=False)

    # ---------------- DRAM parameters ----------------
    xq_d = nc.declare_dram_parameter("xq", [TQ, E], f32, isOutput=False)
    xc_d = nc.declare_dram_parameter("xc", [TCTX, E], f32, isOutput=False)
    memT_d = nc.declare_dram_parameter("memT", [P, KP, 2, TM], f8, isOutput=False)
    w8_names = ["s_wq", "s_wk", "s_wv", "c_wq", "c_wk", "c_wv"]
    w8d = {n: nc.declare_dram_parameter(n, [P, KP, 2, E], f8, isOutput=False)
           for n in w8_names}
    sowP_d = nc.declare_dram_parameter("s_owP", [P, HP, E], bf16, isOutput=False)
    cowP_d = nc.declare_dram_parameter("c_owP", [P, HP, E], bf16, isOutput=False)
    w1T_d = nc.declare_dram_parameter("w1T", [E, F], bf16, isOutput=False)
    w2T_d = nc.declare_dram_parameter("w2T", [F, E], bf16, isOutput=False)
    cmask_d = nc.declare_dram_parameter("cmask", [P, TC], f32, isOutput=False)
    out_d = nc.declare_dram_parameter("out", [TQ, E], f32, isOutput=True)

    xq_r = xq_d.rearrange("(c p) e -> p c e", p=P)        # [128, TQT, E]
    xc_r = xc_d.rearrange("(c p) e -> p c e", p=P)        # [128, TC, E]
    w1T_r = w1T_d.rearrange("(c p) m -> p c m", p=P)      # [128, EC, F]
    w2T_r = w2T_d.rearrange("(c p) m -> p c m", p=P)      # [128, FC, E]
    out_r = out_d.rearrange("(c p) e -> p c e", p=P)

    ctx = ExitStack()
    with ctx:
        ctx.enter_context(nc.allow_low_precision(reason="fp8/bf16 matmul intended"))
        tc = ctx.enter_context(tile.TileContext(nc))

        # ---- kernel-lifetime pools ----
        const = ctx.enter_context(tc.tile_pool(name="const", bufs=1))
        xpool = ctx.enter_context(tc.tile_pool(name="x", bufs=1))
        wq8pool = ctx.enter_context(tc.tile_pool(name="w8", bufs=6))
        cwkvpool = ctx.enter_context(tc.tile_pool(name="cwkv", bufs=4))
        owpool = ctx.enter_context(tc.tile_pool(name="ow", bufs=2))
        ffn1pool = ctx.enter_context(tc.tile_pool(name="ffn1", bufs=3))
        spool = ctx.enter_context(tc.tile_pool(name="stats", bufs=6))
        rdtpool = ctx.enter_context(tc.tile_pool(name="rdt", bufs=3))
        rdpool = ctx.enter_context(tc.tile_pool(name="rd", bufs=1))
        # psum: 5 + 3 = 8 banks
        ps5 = ctx.enter_context(tc.tile_pool(name="ps5", bufs=4, space="PSUM"))
        psS = ctx.enter_context(tc.tile_pool(name="psS", bufs=2, space="PSUM"))

        # ---- constants ----
        identb = const.tile([P, P], bf16)
        make_identity(nc, identb)
        ones_r = const.tile([1, Dh], bf16)
        nc.vector.memset(ones_r[:], 1.0 / WS)
        onesv = const.tile([P, CT, H], f8)
        nc.vector.memset(onesv[:], 1.0)
        onesq = const.tile([P, TQ], f8)
        if uniform_self:
            nc.vector.memset(onesq[:], 1.0)
        eps_t = const.tile([P, 1], f32)
        nc.vector.memset(eps_t[:], EPS)
        cmask_t = const.tile([P, TC], f32)
        if need_mask:
            nc.sync.dma_start(out=cmask_t[:], in_=cmask_d[:])

        if loop_n:
            ctx.enter_context(tc.For_i(0, loop_n, 1))

        # ---- persistent residual state ----
        x_q = xpool.tile([P, TQT, E], f32, tag="xq")
        nc.sync.dma_start(out=x_q[:], in_=xq_r[:])

        # ---------------- helpers ----------------
        def load_w8h(name, half, nm, pool):
            """fp8 DR weight half: [128, KP, 2, 512] (cols half*512...)."""
            t = pool.tile([P, KP, 2, 512], f8, tag="w8", name=nm)
            nc.sync.dma_start(
                out=t[:], in_=w8d[name][:, :, :, half * 512:(half + 1) * 512])
            return t

        def load_w8(name, nm, pool=None):
            pool = pool or wq8pool
            return [load_w8h(name, 0, nm + "a", pool),
                    load_w8h(name, 1, nm + "b", pool)]

        def w8col(wpair, c0):
            """[128, 2, 128] DR stationary slice at out-col c0 (needs kp idx)."""
            h, off = divmod(c0, 512)
            return wpair[h], off

        def layernorm(x_tiles, n_tiles, out_tile):
            """token-major LN: out = (x - mean) * rsqrt(var + eps)."""
            for t in range(n_tiles):
                st = spool.tile([P, 2, 6], f32, tag="bnst")
                xin = x_tiles[:, t, :].rearrange("p (s e) -> p s e", s=2)
                for s in range(2):
                    nc.vector.bn_stats(out=st[:, s, :], in_=xin[:, s, :])
                mv = spool.tile([P, 2], f32, tag="bnmv")
                nc.vector.bn_aggr(out=mv[:], in_=st[:])
                sd = spool.tile([P, 1], f32, tag="bnsd")
                nc.scalar.activation(sd[:], mv[:, 1:2], AF.Sqrt, bias=eps_t[:])
                rstd = spool.tile([P, 1], f32, tag="bnrs")
                nc.vector.reciprocal(rstd[:], sd[:])
                nm = spool.tile([P, 1], f32, tag="bnnm")
                nc.vector.tensor_mul(nm[:], mv[:, 0:1], rstd[:])
                nc.scalar.mul(nm[:], nm[:], -1.0)
                nc.scalar.activation(out_tile[:, t, :], x_tiles[:, t, :], AF.Identity,
                                     bias=nm[:], scale=rstd[:])

        def transpose_in(nx_tiles, n_tiles, outT, ident):
            """(128, n_tiles, E) token-major -> (128, EC, n_tiles*128) feat-major.
            dtype of nx/outT/ident must match (fp8 or bf16)."""
            dt_ = outT.dtype if hasattr(outT, "dtype") else f8
            for e in range(EC):
                for t0 in range(0, n_tiles, 4):
                    tn = min(4, n_tiles - t0)
                    pst = ps5.tile([P, 512], ident.dtype, tag="ps5")
                    for i in range(tn):
                        nc.tensor.transpose(
                            pst[:, i * P:(i + 1) * P],
                            nx_tiles[:, t0 + i, e * P:(e + 1) * P], ident[:])
                    nc.scalar.activation(outT[:, e, t0 * P:(t0 + tn) * P],
                                         pst[:, 0:tn * P], AF.Copy)

        def proj_dr(src_pair, src_cols, wpair, outT, eng="scalar"):
            """Feature-major DR projection.
            src_pair(kp, cols) -> [128, 2, cols] moving slice for feat-pair kp.
            outT[:, m, :cols] = (w.T x)."""
            for m in range(EC):
                ps = ps5.tile([P, 512], f32, tag="ps5")
                wh, off = w8col(wpair, m * P)
                for kp in range(KP):
                    nc.tensor.matmul(ps[:, 0:src_cols],
                                     wh[:, kp, :, off:off + P],
                                     src_pair(kp, src_cols),
                                     start=(kp == 0), stop=(kp == KP - 1),
                                     perf_mode=DR)
                copy_out(eng, outT[:, m, 0:src_cols], ps[:, 0:src_cols])

        def copy_out(eng, dst, src):
            if eng == "scalar":
                nc.scalar.activation(dst, src, AF.Copy)
            else:
                nc.vector.tensor_copy(dst, src)

        def kv_project_gen(srcT_pair, src_tok_pair, c0, ct, wk, wv, kTc, vpk,
                           dst0=0, eng="vector"):
            """Generator: one PSUM group per next(). K into kTc (bf16
            feat-major) at col dst0, V into vpk (fp8 token-major + ones col)
            at tile dst0//P.
            srcT_pair(kp, c0, n) -> [128, 2, n] moving (features x tokens);
            src_tok_pair(kp, t0) -> [128, 2, 128] stationary (feature x token)."""
            dt0 = dst0 // P
            nc.vector.tensor_copy(vpk[:, dt0:dt0 + ct, :, 64], onesv[:, 0:ct, :])
            for m in range(EC):
                ps = ps5.tile([P, 512], f32, tag="ps5")
                wh, off = w8col(wk, m * P)
                for kp in range(KP):
                    nc.tensor.matmul(ps[:, 0:ct * P],
                                     wh[:, kp, :, off:off + P],
                                     srcT_pair(kp, c0, ct * P),
                                     start=(kp == 0), stop=(kp == KP - 1),
                                     perf_mode=DR)
                copy_out(eng, kTc[:, m, dst0:dst0 + ct * P], ps[:, 0:ct * P])
                yield
            for mt in range(ct):
                for half in range(2):
                    ps = ps5.tile([P, 512], f32, tag="ps5")
                    for kp in range(KP):
                        nc.tensor.matmul(
                            ps[:],
                            src_tok_pair(kp, c0 + mt * P),
                            wv[half][:, kp, :, :],
                            start=(kp == 0), stop=(kp == KP - 1),
                            perf_mode=DR)
                    copy_out(eng,
                             vpk[:, dt0 + mt, half * 8:(half + 1) * 8, 0:64],
                             ps.rearrange("p (h d) -> p h d", d=64))
                    yield

        def kv_project(*args, **kwargs):
            for _ in kv_project_gen(*args, **kwargs):
                pass

        def attn_chunk(qT, kTc, vpk, ct, acc65, ppool, first, last, masked,
                       uniform, kc0=0, filler=None):
            """One kv chunk of attention for all heads; accumulate into acc65.
            kc0: column offset into kTc (and tile offset kc0//P into vpk).
            filler(h): emits extra PE work between scores and AV."""
            vt0 = kc0 // P
            for h in range(H):
                ht, hr = h // 2, (h % 2) * Dh
                probsT = ppool.tile([P, CT, TQ], f8, tag="probs")
                if not uniform:
                    ps_s = psS.tile([P, CT, TQ], f32, tag="psS")
                    for kt in range(ct):
                        nc.tensor.matmul(
                            ps_s[:, kt, :],
                            kTc[hr:hr + Dh, ht, kc0 + kt * P:kc0 + (kt + 1) * P],
                            qT[hr:hr + Dh, ht, :],
                            start=True, stop=True)
                    nc.scalar.activation(probsT[:, 0:ct, :],
                                         ps_s[:, 0:ct, :], AF.Exp,
                                         scale=EXP_SCALE)
                    if masked and last:
                        nc.vector.tensor_scalar_mul(
                            probsT[:, ct - 1, :], probsT[:, ct - 1, :],
                            cmask_t[:, TC - 1:TC])
                else:
                    for kt in range(ct):
                        nc.vector.tensor_copy(probsT[:, kt, :], onesq[:])
                if filler is not None:
                    filler(h)
                ps_av = ps5.tile([65, TQ], f32, tag="ps5")
                nkt = ct // 2
                for ktp in range(nkt):
                    nc.tensor.matmul(ps_av[:],
                                     vpk[:, vt0 + 2 * ktp:vt0 + 2 * ktp + 2, h, :],
                                     probsT[:, 2 * ktp:2 * ktp + 2, :],
                                     start=(ktp == 0),
                                     stop=(ktp == nkt - 1 and ct % 2 == 0),
                                     perf_mode=DR)
                if ct % 2:
                    nc.tensor.matmul(ps_av[:],
                                     vpk[:, vt0 + ct - 1, h, :],
                                     probsT[:, ct - 1, :],
                                     start=(ct == 1), stop=True)
                if first:
                    nc.vector.tensor_copy(acc65[:, h, :], ps_av[:])
                else:
                    nc.vector.tensor_add(acc65[:, h, :], acc65[:, h, :], ps_av[:])

        def attn_super(qT, kv0, kv1, acc65, ppool, first, filler=None):
            """Two cross kv chunks per acc65 update (CT tiles each, no mask)."""
            kTc0, vpk0 = kv0
            kTc1, vpk1 = kv1
            for h in range(H):
                ht, hr = h // 2, (h % 2) * Dh
                probs = []
                for kTc in (kTc0, kTc1):
                    probsT = ppool.tile([P, CT, TQ], f8, tag="probs")
                    ps_s = psS.tile([P, CT, TQ], f32, tag="psS")
                    for kt in range(CT):
                        nc.tensor.matmul(
                            ps_s[:, kt, :],
                            kTc[hr:hr + Dh, ht, kt * P:(kt + 1) * P],
                            qT[hr:hr + Dh, ht, :],
                            start=True, stop=True)
                    nc.scalar.activation(probsT[:, 0:CT, :],
                                         ps_s[:, 0:CT, :], AF.Exp,
                                         scale=EXP_SCALE)
                    if filler is not None:
                        filler(h)
                    probs.append(probsT)
                ps_av = ps5.tile([65, TQ], f32, tag="ps5")
                nsub = CT // 2
                for sub in range(2):
                    vpk, probsT = (vpk0, probs[0]) if sub == 0 else (vpk1, probs[1])
                    for ktp in range(nsub):
                        nc.tensor.matmul(ps_av[:],
                                         vpk[:, 2 * ktp:2 * ktp + 2, h, :],
                                         probsT[:, 2 * ktp:2 * ktp + 2, :],
                                         start=(sub == 0 and ktp == 0),
                                         stop=(sub == 1 and ktp == nsub - 1),
                                         perf_mode=DR)
                if first:
                    nc.vector.tensor_copy(acc65[:, h, :], ps_av[:])
                else:
                    nc.vector.tensor_add(acc65[:, h, :], acc65[:, h, :], ps_av[:])

        def normalize(acc65, nm):
            """acc65 [65, H, TQ] -> accP [128, HP, TQ] bf16 pair-packed
            (odd heads land in partitions 64-127 via SBUF->SBUF DMA).
            Folds the 1/WS V-scale via ones_r."""
            accP = rdpool.tile([P, HP, TQ], bf16, tag="accb", name=f"{nm}ab")
            rds = []
            for t in range(HP):
                rd = rdtpool.tile([1, 2, TQ], bf16, tag="rdt", name=f"{nm}rd{t}")
                nc.vector.reciprocal(rd[:], acc65[64:65, 2 * t:2 * t + 2, :])
                rds.append(rd)
            for t in range(HP):
                psb = psS.tile([Dh, 2, TQ], f32, tag="psS", name=f"{nm}psb{t}")
                nc.tensor.matmul(psb[:], ones_r[:], rds[t][:],
                                 start=True, stop=True)
                nc.vector.tensor_mul(accP[0:64, t, :],
                                     acc65[0:64, 2 * t, :], psb[:, 0, :])
                odd = rdtpool.tile([Dh, TQ], bf16, tag="odd", name=f"{nm}od{t}")
                nc.vector.tensor_mul(odd[:],
                                     acc65[0:64, 2 * t + 1, :], psb[:, 1, :])
                nc.sync.dma_start(out=accP[64:128, t, :], in_=odd[:])
            return accP

        def out_proj(accP, owP_dram, dest, nm):
            """dest[:, tq, :] += attn @ ow; head-pairs packed, K=128."""
            for nq in range(4):
                owq = owpool.tile([P, HP, 256], bf16, tag="ow", name=f"{nm}{nq}")
                nc.sync.dma_start(out=owq[:],
                                  in_=owP_dram[:, :, nq * 256:(nq + 1) * 256])
                for tq in range(TQT):
                    ps = psS.tile([P, TQ], f32, tag="psS")
                    for hp in range(HP):
                        nc.tensor.matmul(
                            ps[:],
                            accP[:, hp, tq * P:(tq + 1) * P],
                            owq[:, hp, :],
                            start=(hp == 0), stop=(hp == HP - 1))
                    nc.vector.tensor_add(dest[:, tq, nq * 256:(nq + 1) * 256],
                                         dest[:, tq, nq * 256:(nq + 1) * 256],
                                         ps[:])

        # =========================================================
        # Phase 1: LN1 + transposes (fp8), self QKV, cross KV c0/c1
        mpool = ctx.enter_context(tc.tile_pool(name="mem", bufs=3))
        kTpool = ctx.enter_context(tc.tile_pool(name="kT", bufs=3))
        vpool = ctx.enter_context(tc.tile_pool(name="v", bufs=3))

        cross_kv = {}

        def emit_cross_kv(c, eng="vector"):
            mt_ = mpool.tile([P, KP, 2, CHUNK], f8, tag="mem", name=f"mem{c}")
            nc.sync.dma_start(
                out=mt_[:],
                in_=memT_d[:, :, :, c * CHUNK:(c + 1) * CHUNK])
            kTc = kTpool.tile([P, EC, CHUNK], bf16, tag="kT", name=f"ckT{c}")
            vpk = vpool.tile([P, CT, H, 65], f8, tag="v", name=f"cv{c}")
            cross_kv[c] = (kTc, vpk)
            return kv_project_gen(
                lambda kp, c0, n: mt_[:, kp, :, c0:c0 + n],
                lambda kp, t0: mt_[:, kp, :, t0:t0 + P],
                0, CT, cwk, cwv, kTc, vpk, eng=eng)

        swq = load_w8("s_wq", "swq")
        with tc.tile_pool(name="nxT1", bufs=1) as nxT1:
            nxqT = nxT1.tile([P, EC, TQ], f8, tag="nxqT")
            nxcT = nxT1.tile([P, EC, TCTX], f8, tag="nxcT")
            with tc.tile_pool(name="p1", bufs=1) as p1:
                xc_t = p1.tile([P, TC, E], f32, tag="xc")
                nc.sync.dma_start(out=xc_t[:], in_=xc_r[:])
                nx_q = p1.tile([P, TQT, E], bf16, tag="nxq")
                layernorm(x_q, TQT, nx_q)
                transpose_in(nx_q, TQT, nxqT, identb)
                nx_c = p1.tile([P, TC, E], bf16, tag="nxc")
                layernorm(xc_t, TC, nx_c)
                transpose_in(nx_c, TC, nxcT, identb)

            # Phase 2: self QKV + attention + out_proj
            cwk = load_w8("c_wk", "cwk", cwkvpool)
            cwv = load_w8("c_wv", "cwv", cwkvpool)
            with tc.tile_pool(name="qT2", bufs=1) as qTp, \
                 tc.tile_pool(name="skT", bufs=1) as skTp, \
                 tc.tile_pool(name="sv", bufs=1) as svp, \
                 tc.tile_pool(name="pr2", bufs=3) as pp, \
                 tc.tile_pool(name="at2", bufs=1) as ap_:
                qT = qTp.tile([P, EC, TQ], bf16, tag="qT")
                proj_dr(lambda kp, n: nxqT[:, 2 * kp:2 * kp + 2, 0:n], TQ,
                        swq, qT)
                swk = load_w8("s_wk", "swk")
                swv = load_w8("s_wv", "swv")
                kTc_s = skTp.tile([P, EC, TCTX], bf16, tag="skT")
                vpk_s = svp.tile([P, TC, H, 65], f8, tag="sv")
                # self K/V over the context tiles (chunks of <=CT tiles)
                c0 = 0
                while c0 < TCTX:
                    ct = min(CT, (TCTX - c0) // P)
                    kv_project(
                        lambda kp, cc, n: nxcT[:, 2 * kp:2 * kp + 2, cc:cc + n],
                        lambda kp, t0: nxcT[:, 2 * kp:2 * kp + 2, t0:t0 + P],
                        c0, ct, swk, swv, kTc_s, vpk_s, dst0=c0)
                    c0 += ct * P
                # cross kv chunk 0 interleaved into self attention
                kv0 = emit_cross_kv(0)
                acc65 = ap_.tile([65, H, TQ], f32r, tag="acc65")
                n_sc = (TCTX + CHUNK - 1) // CHUNK
                for ci in range(n_sc):
                    cc0 = ci * CHUNK
                    ct = min(CT, (TCTX - cc0) // P)
                    attn_chunk(qT, kTc_s, vpk_s, ct, acc65, pp,
                               ci == 0, ci == n_sc - 1, need_mask, uniform_self,
                               kc0=cc0,
                               filler=(lambda h: next(kv0, None)) if ci == 0
                               else None)
                for _ in kv0:
                    pass
                for _ in emit_cross_kv(1, eng="scalar"):
                    pass
                accb_s = normalize(acc65, "s")
                out_proj(accb_s, sowP_d, x_q, "sow")

        # Phase 3/4: LN2 + transpose, cross q + attention + out_proj
        with tc.tile_pool(name="qT4", bufs=1) as qTp, \
             tc.tile_pool(name="pr4", bufs=3) as pp, \
             tc.tile_pool(name="at4", bufs=1) as ap_:
            with tc.tile_pool(name="nxT3", bufs=1) as nxT3:
                nx2T = nxT3.tile([P, EC, TQ], f8, tag="nx2T")
                with tc.tile_pool(name="p3", bufs=1) as p3:
                    nx2 = p3.tile([P, TQT, E], bf16, tag="nx2")
                    layernorm(x_q, TQT, nx2)
                    transpose_in(nx2, TQT, nx2T, identb)
                cwq = load_w8("c_wq", "cwq")
                qT = qTp.tile([P, EC, TQ], bf16, tag="qT")
                proj_dr(lambda kp, n: nx2T[:, 2 * kp:2 * kp + 2, 0:n], TQ,
                        cwq, qT, eng="vector")
            acc65 = ap_.tile([65, H, TQ], f32r, tag="acc65")
            n_cc = TM // CHUNK
            for c in range(n_cc):
                gen = emit_cross_kv(c + 2) if c + 2 < n_cc else None
                if c == 1:
                    for g in range(2):
                        w1g = ffn1pool.tile([P, EC, 512], bf16, tag="ffn1",
                                            name=f"w1g{g}")
                        nc.sync.dma_start(
                            out=w1g[:], in_=w1T_r[:, :, g * 512:(g + 1) * 512])
                        cross_kv[f"w1_{g}"] = w1g
                kTc, vpk = cross_kv.pop(c)
                attn_chunk(qT, kTc, vpk, CT, acc65, pp,
                           c == 0, c == n_cc - 1, False, False,
                           filler=(lambda h, g=gen: next(g, None))
                           if gen is not None else None)
                if gen is not None:
                    for _ in gen:
                        pass
            accb_c = normalize(acc65, "c")
            out_proj(accb_c, cowP_d, x_q, "cow")

        # Phase 5: LN3 + transpose (bf16); Phase 6: FFN
        with tc.tile_pool(name="nxT5", bufs=1) as nxT5:
            nx3T = nxT5.tile([P, EC, TQ], bf16, tag="nx3T")
            with tc.tile_pool(name="p5", bufs=1) as p5:
                nx3 = p5.tile([P, TQT, E], bf16, tag="nx3")
                layernorm(x_q, TQT, nx3)
                transpose_in(nx3, TQT, nx3T, identb)

            with tc.tile_pool(name="hT", bufs=1) as hp_, \
                 tc.tile_pool(name="ffn2", bufs=3) as ffn2pool:
                hT = hp_.tile([P, FC, TQ], bf16, tag="hT")
                w2pre = {}
                for g in range(2):
                    w2g = ffn2pool.tile([P, 4, E], bf16, tag="ffn2",
                                        name=f"w2g{g}")
                    nc.sync.dma_start(out=w2g[:],
                                      in_=w2T_r[:, g * 4:(g + 1) * 4, :])
                    w2pre[g] = w2g
                for g in range(8):
                    if f"w1_{g}" in cross_kv:
                        w1g = cross_kv.pop(f"w1_{g}")
                    else:
                        w1g = ffn1pool.tile([P, EC, 512], bf16, tag="ffn1",
                                            name=f"w1g{g}")
                        nc.sync.dma_start(
                            out=w1g[:], in_=w1T_r[:, :, g * 512:(g + 1) * 512])
                    for m in range(4):
                        ps = ps5.tile([P, 512], f32, tag="ps5")
                        for k in range(EC):
                            nc.tensor.matmul(ps[:, 0:TQ],
                                             w1g[:, k, m * P:(m + 1) * P],
                                             nx3T[:, k, :],
                                             start=(k == 0), stop=(k == EC - 1))
                        nc.scalar.activation(hT[:, g * 4 + m, :], ps[:, 0:TQ],
                                             AF.Gelu)
                # y = hT.T @ w2T, accumulated over all 32 k-tiles
                ps_y = [[ps5.tile([P, 512], f32, tag="ps5", name=f"psy_{tq}_{nh}")
                         for nh in range(2)] for tq in range(TQT)]
                for g in range(8):
                    if g in w2pre:
                        w2g = w2pre.pop(g)
                    else:
                        w2g = ffn2pool.tile([P, 4, E], bf16, tag="ffn2",
                                            name=f"w2g{g}")
                        nc.sync.dma_start(out=w2g[:],
                                          in_=w2T_r[:, g * 4:(g + 1) * 4, :])
                    for tq in range(TQT):
                        for nh in range(2):
                            for k in range(4):
                                kk = g * 4 + k
                                nc.tensor.matmul(
                                    ps_y[tq][nh][:],
                                    hT[:, kk, tq * P:(tq + 1) * P],
                                    w2g[:, k, nh * 512:(nh + 1) * 512],
                                    start=(kk == 0), stop=(kk == FC - 1))
                for tq in range(TQT):
                    for nh in range(2):
                        nc.vector.tensor_add(x_q[:, tq, nh * 512:(nh + 1) * 512],
                                             x_q[:, tq, nh * 512:(nh + 1) * 512],
                                             ps_y[tq][nh][:])

        nc.sync.dma_start(out=out_r[:], in_=x_q[:])

    nc.finalize()
    return nc


# ======================= host side =======================

def host_prep(inputs):
    """Fold LN affine into weights; fp8/bf16 layouts; slice per core.
    Returns (n_ctx, in_maps list of 8 dicts)."""
    import ml_dtypes
    F8 = ml_dtypes.float8_e4m3
    BF = ml_dtypes.bfloat16

    tgt = np.asarray(inputs["tgt"], np.float32)
    memory = np.asarray(inputs["memory"], np.float32)
    n_ctx = int(np.asarray(inputs["n_ctx"]))
    ln1_g = np.asarray(inputs["ln1_g"], np.float32)
    ln1_b = np.asarray(inputs["ln1_b"], np.float32)
    ln2_g = np.asarray(inputs["ln2_g"], np.float32)
    ln2_b = np.asarray(inputs["ln2_b"], np.float32)
    ln3_g = np.asarray(inputs["ln3_g"], np.float32)
    ln3_b = np.asarray(inputs["ln3_b"], np.float32)

    def fold(w, b, g, lb):
        w_eff = w * g[None, :]
        b_eff = w @ lb + b
        return w_eff, b_eff

    s_w = np.asarray(inputs["self_w"], np.float32)
    s_b = np.asarray(inputs["self_b"], np.float32)
    c_w = np.asarray(inputs["cross_w"], np.float32)
    c_b = np.asarray(inputs["cross_b"], np.float32)
    s_ow = np.asarray(inputs["self_ow"], np.float32)
    s_ob = np.asarray(inputs["self_ob"], np.float32)
    c_ow = np.asarray(inputs["cross_ow"], np.float32)
    c_ob = np.asarray(inputs["cross_ob"], np.float32)
    w1 = np.asarray(inputs["w1"], np.float32)
    b1 = np.asarray(inputs["b1"], np.float32)
    w2 = np.asarray(inputs["w2"], np.float32)
    b2 = np.asarray(inputs["b2"], np.float32)

    s_wq, s_bq = fold(s_w[:E], s_b[:E], ln1_g, ln1_b)
    s_wk, s_bk = fold(s_w[E:2 * E], s_b[E:2 * E], ln1_g, ln1_b)
    s_wv, s_bv = fold(s_w[2 * E:], s_b[2 * E:], ln1_g, ln1_b)
    c_wq, c_bq = fold(c_w[:E], c_b[:E], ln2_g, ln2_b)
    c_wk, c_bk = c_w[E:2 * E], c_b[E:2 * E]
    c_wv, c_bv = c_w[2 * E:], c_b[2 * E:]
    w1_eff, b1_eff = fold(w1, b1, ln3_g, ln3_b)

    zero_bias = not (np.any(s_bq) or np.any(s_bk) or np.any(s_bv) or np.any(s_ob)
                     or np.any(c_bq) or np.any(c_bk) or np.any(c_bv) or np.any(c_ob)
                     or np.any(b1_eff) or np.any(b2))
    if not zero_bias:
        raise NotImplementedError("nonzero biases not supported by this kernel")

    uniform = n_ctx == 0
    n_ctx_eff = 1024 if uniform else n_ctx
    TC = (n_ctx_eff + P - 1) // P
    TCTX = TC * P

    def w8dr(w):
        """[E_out, E_in] weight -> fp8 DR layout [128, KP, 2, E_out], x WS."""
        wT = np.ascontiguousarray(w.T) * WS                  # [E_in, E_out]
        return np.ascontiguousarray(
            wT.reshape(KP, 2, P, w.shape[0]).transpose(2, 0, 1, 3)).astype(F8)

    def owpair(ow):
        """[E, E] out-proj -> bf16 pair layout [128, HP, E]."""
        owT = np.ascontiguousarray(ow.T)                     # [E_in(hd), E_out]
        return np.ascontiguousarray(
            owT.reshape(HP, 2, Dh, E).transpose(1, 2, 0, 3).reshape(P, HP, E)
        ).astype(BF)

    def b16(a):
        return np.ascontiguousarray(a.T).astype(BF)

    def memdr(m):
        """[Tk, E] memory -> fp8 DR layout [128, KP, 2, Tk]."""
        mT = np.ascontiguousarray(m.T)                       # [E, Tk]
        return np.ascontiguousarray(
            mT.reshape(KP, 2, P, m.shape[0]).transpose(2, 0, 1, 3)).astype(F8)

    shared = {
        "s_wq": w8dr(s_wq),
        "s_wk": w8dr(s_wk),
        "s_wv": w8dr(s_wv),
        "c_wq": w8dr(c_wq),
        "c_wk": w8dr(c_wk),
        "c_wv": w8dr(c_wv),
        "s_owP": owpair(s_ow),
        "c_owP": owpair(c_ow),
        "w1T": b16(w1_eff),
        "w2T": b16(w2),
    }
    cmask = np.ones((P, TC), np.float32)
    nvalid = n_ctx_eff - (TC - 1) * P
    cmask[nvalid:, TC - 1] = 0.0

    memT = [memdr(memory[g]) for g in range(2)]
    xcs = []
    for g in range(2):
        xc = tgt[g, :min(n_ctx_eff, 1024)]
        if xc.shape[0] < TCTX:
            xc = np.concatenate([xc, np.zeros((TCTX - xc.shape[0], E), np.float32)], 0)
        xcs.append(np.ascontiguousarray(xc))

    in_maps = []
    for c in range(8):
        g, r0 = c // 4, (c % 4) * TQ
        m = dict(shared)
        m["memT"] = memT[g]
        m["xq"] = np.ascontiguousarray(tgt[g, r0:r0 + TQ])
        m["xc"] = xcs[g]
        m["cmask"] = cmask
        in_maps.append(m)
    return n_ctx, in_maps


def assemble(results):
    out = np.empty((2, 1024, E), np.float32)
    for c in range(8):
        g, r0 = c // 4, (c % 4) * TQ
        out[g, r0:r0 + TQ] = results[c]["out"]
    return out


_NC_CACHE = {}


def kernel(**inputs):
    """Full (unsharded) inputs -> full (2, 1024, 1024) float32 output."""
    from concourse.bass_utils import run_bass_kernel_spmd
    n_ctx, in_maps = host_prep(inputs)
    nc = _NC_CACHE.get(n_ctx)
    if nc is None:
        nc = build_nc(n_ctx)
        _NC_CACHE[n_ctx] = nc
    res = run_bass_kernel_spmd(nc, in_maps, list(range(8)))
    return assemble(res.results)
